# revision 19
# baseline (speedup 1.0000x reference)
"""Trainium2 Bass kernel for a bidirectional GRU language model head.

Model (see problem reference): tokens x[T=64, B=64] -> embedding[32000, 32]
-> forward GRU (H=8, scalar z/r gates) + backward GRU -> concat [T,B,16]
-> logits = h @ Wout[16, 32000] + bout -> log_softmax over vocab.

Sharding: data-parallel over batch; core c gets batch columns [8c, 8c+8)
and runs the full T=64 recurrence plus the full-vocab projection for its
512 tokens. No collectives. Output is written bf16 (rel err ~4e-3 vs the
2e-2 gate) and widened to f32 on the host during the unshard, halving
the dominant HBM write traffic.

Device plan per core:
  1. Embedding gather per 128-token group in readiness order (0,3,1,2):
     groups 0/3 up front (the scan's first steps touch both sequence
     ends), groups 1/2 emitted INTO the scan so their ~9us indirect-DMA
     latencies hide under early scan steps.  Each group: indirect DMA ->
     PE transpose -> P20 slab (input-side gate terms, biases folded).
  2. GRU scans (both directions interleaved in one [98, BS] f16 state),
     63 dependent steps.  f16 weights/state avoid f32's two half-speed
     matmul lowering.  The z/r weight columns are replicated 32x so
     sigmoid(zr matmul) IS the broadcast gate tile (no stream_shuffle).
     Next-step e-term refresh copies ride gpsimd (f32->f16 cast) so the
     DVE stream holds only the r-path/update ops.  Pre-update states are
     cast (f16->bf16) by gpsimd straight into compact projection lhsT
     tiles.
  3. Projection, single pass, software-pipelined across the four
     128-token tiles in readiness order (1,2,0,3).  Per 4096-column
     double-unit: 8 bf16 matmuls (K=17 compact weights, vocab padded to
     32768 with -40 bias so pad columns vanish under exp) -> two drains
     PSUM -> bf16 SBUF stage (DVE 10/16, ACT 6/16) -> ONE ACT
     exp+accumulate over the staged 4096 columns (out dumped to SBUF
     scratch so PSUM frees at drain time).  After a tile's 8 double
     units: logsumexp; each double-unit then gets one in-place DVE
     tensor_scalar add (all-bf16 SBUF high-perf mode) and its output
     DMA.  Tile t's finals interleave with tile t+1's units so ACT(exp),
     DVE(drain+final), PE and the output DMA all run concurrently, and
     all projection ACT funcs (Exp, Ln, Copy) share one table set.
"""

import numpy as np
import ml_dtypes

VOCAB, HID, EMB = 32000, 8, 32
VPAD = 32768                  # vocab padded to 8 double-units of 4096
SEQ, BATCH = 64, 64
NCORES = 8
BS = BATCH // NCORES          # batch columns per core
TOK = SEQ * BS                # tokens per core
NCHUNK = 512                  # vocab columns per matmul = one PSUM bank
UCH = 4                       # chunks per PSUM tile / drain
DCOL = 2 * UCH * NCHUNK       # 4096 columns per double-unit (one exp)

_module_cache = {}


def _build_module(vocab=VOCAB, proj_order=(1, 2, 0, 3)):
    import concourse.bass as bass
    import concourse.bacc as bacc
    import concourse.mybir as mybir
    import concourse.tile as tile
    from concourse.masks import make_identity

    dt = mybir.dt
    AF = mybir.ActivationFunctionType

    dunits = VPAD // DCOL     # 8 double-units per tile
    NT = TOK // 128           # 128-token projection tiles (4)

    nc = bacc.Bacc("TRN2", target_bir_lowering=False, debug=False)

    # Scan-gating inputs ship TRANSPOSED and 2-byte so they load via the
    # DMA transpose XBAR: a handful of large contiguous DRAM descriptors
    # instead of one tiny descriptor per partition (which serializes to
    # ~40us on one DMA engine).  idx[p, g] = x[g*16 + p//8, p%8], int16
    # (vocab < 32768), shipped as [16, 128] with rows 4:16 zero-padded.
    x_d = nc.dram_tensor("x", [16, 128], dt.int16, kind="ExternalInput")
    emb_d = nc.dram_tensor("emb", [vocab, EMB], dt.float32, kind="ExternalInput")
    wea_d = nc.dram_tensor("wea", [EMB + 1, 104], dt.float32, kind="ExternalInput")
    wzr_d = nc.dram_tensor("wzr", [128, 128], dt.float16, kind="ExternalInput")
    whh_d = nc.dram_tensor("whh", [64, 128], dt.float16, kind="ExternalInput")
    wout_d = nc.dram_tensor("wout", [17, VPAD], dt.bfloat16, kind="ExternalInput")
    out_d = nc.dram_tensor("out", [TOK, vocab], dt.bfloat16, kind="ExternalOutput")

    with tile.TileContext(nc) as tc:
        with (
            tc.tile_pool(name="const", bufs=1) as cpool,
            tc.tile_pool(name="scan", bufs=2) as spool,
            tc.tile_pool(name="stage", bufs=11) as stgp,
            tc.tile_pool(name="gath", bufs=2) as gpool,
        ):
            # ---- constants / inputs to SBUF ----
            idx16 = cpool.tile([128, 16], dt.int16)
            nc.sync.dma_start(idx16[:], x_d[:], transpose=True)
            idx_sb = cpool.tile([128, NT], dt.int32)
            nc.vector.tensor_copy(idx_sb[:], idx16[:, 0:NT])
            wzrT = cpool.tile([128, 128], dt.float16)
            nc.sync.dma_start(wzrT[:], wzr_d[:], transpose=True)
            wzr_sb = wzrT[0:98, :]
            whhT = cpool.tile([128, 64], dt.float16)
            nc.sync.dma_start(whhT[:], whh_d[:], transpose=True)
            whh_sb = whhT[0:64, :]
            wea_sb = cpool.tile([EMB + 1, 104], dt.float32)
            nc.sync.dma_start(wea_sb[:], wea_d[:])
            wout_sb = cpool.tile([17, VPAD], dt.bfloat16)
            nc.scalar.dma_start(wout_sb[:], wout_d[:])
            ident_sb = cpool.tile([128, 128], dt.float32)
            make_identity(nc, ident_sb[:])

            encT = cpool.tile([EMB + 1, TOK], dt.float32)
            nc.vector.memset(encT[EMB : EMB + 1, :], 1.0)
            # P20 rows (quadrant-aligned): 0:2 = z1,r1; 32:34 = z2,r2;
            # 64:72 = h1e; 96:104 = h2e.  Biases folded via encT ones row.
            P20 = cpool.tile([104, TOK], dt.float32)
            # P20EH [64, TOK]: rows 0:8 = h1e in token order; rows 32:40 = h2e
            # in REVERSED block order (block j holds e-terms of t = 63-j), so a
            # single [64]-row add serves both scan directions each step.
            P20EH = cpool.tile([64, TOK], dt.float32)
            nc.vector.memset(P20EH[:], 0.0)
            # compact projection lhsT: rows 0:8 fwd h, 8:16 bwd h, 16 ones.
            # Scan stores land in HTf (rows 0:8 directly) and HTbk (bwd, a
            # 0-based tile; compute APs must start at partition 0/32/64/96,
            # DMA later moves it to rows 8:16).
            HTf = [cpool.tile([17, 128], dt.bfloat16, name=f"HTf{m}", tag=f"HTf{m}")
                   for m in range(NT)]
            HTbk = [cpool.tile([8, 128], dt.bfloat16, name=f"HTbk{m}", tag=f"HTbk{m}")
                    for m in range(NT)]
            for m in range(NT):
                nc.vector.memset(HTf[m][:], 1.0)   # row 16 = bias ones lane
                nc.vector.memset(HTbk[m][:], 0.0)
            nc.vector.memset(HTf[0][0:8, 0:BS], 0.0)  # fwd state 0 @ t=0
            # bwd state 0 @ t=63 is covered by the HTbk zero memset
            sums = [cpool.tile([128, dunits], dt.float32, name=f"sums{m}")
                    for m in range(NT)]
            nlz = [cpool.tile([128, 2], dt.float32, name=f"nlz{m}")
                   for m in range(NT)]

            # ---- phase 1: per-group embedding gather -> encT -> P20 ----
            pstp = tc.alloc_tile_pool(name="pst", bufs=1, space="PSUM")
            zrpsp = tc.alloc_tile_pool(name="zrps", bufs=2, space="PSUM")
            gpsp = tc.alloc_tile_pool(name="gps", bufs=2, space="PSUM")
            p20ps = pstp.tile([104, TOK], dt.float32, tag="p20")

            def group_setup(g):
                c0, c1 = g * 128, (g + 1) * 128
                encg = gpool.tile([128, EMB], dt.float32, tag="encg")
                nc.gpsimd.indirect_dma_start(
                    out=encg[:],
                    out_offset=None,
                    in_=emb_d.ap(),
                    in_offset=bass.IndirectOffsetOnAxis(ap=idx_sb[:, g : g + 1], axis=0),
                )
                pst = pstp.tile([EMB, 128], dt.float32, tag="pst")
                nc.tensor.transpose(out=pst[:], in_=encg[:], identity=ident_sb[:])
                nc.vector.tensor_copy(encT[0:EMB, c0:c1], pst[:])
                nc.tensor.matmul(p20ps[:, c0:c1], lhsT=wea_sb[:],
                                 rhs=encT[:, c0:c1], start=True, stop=True)
                nc.vector.tensor_copy(P20[:, c0:c1], p20ps[:, c0:c1])
                nc.vector.tensor_copy(P20EH[0:8, c0:c1], p20ps[64:72, c0:c1])

            def rev_copies(js, eng):
                # P20EH bwd rows, block j <- e-terms of t = 63-j
                for j in js:
                    eng.tensor_copy(
                        P20EH[32:40, j * BS : (j + 1) * BS],
                        P20[96:104, (SEQ - 1 - j) * BS : (SEQ - j) * BS])

            group_setup(0)
            group_setup(3)
            # rev copies whose source P20 groups (3 and 0) are already queued
            rev_copies(range(0, 16), nc.vector)     # src group 3
            rev_copies(range(48, 64), nc.vector)    # src group 0

            # ---- phase 2: the two GRU scans, interleaved, 63 steps ----
            # state S [98, BS] f16: rows 0:8 fwd h, 32:40 bwd h, 64:66 fwd
            # (ez, er), 96:98 bwd (ez, er).  Selector rows of wzr add the
            # e-terms; wzr columns replicated 32x per gate so sigmoid(zrps)
            # is the broadcast tile bc: rows 0:32 = z1, 32:64 = z2,
            # 64:96 = r1, 96:128 = r2.
            S = spool.tile([98, BS], dt.float16, tag="S")
            nc.vector.memset(S[0:64, :], 0.0)
            # P20 rows 2:32 are zero, so this fills 64:96 with [ez1,er1; 0...]
            nc.vector.tensor_copy(S[64:96, :], P20[0:32, 0:BS])
            nc.vector.tensor_copy(S[96:98, :], P20[32:34, (SEQ - 1) * BS : SEQ * BS])

            for s in range(SEQ - 1):
                if s == 7:
                    # group 1's gather has landed by now; its P20 slab is
                    # needed from step 15 (fwd) / the rev copies from step 32
                    group_setup(1)
                    rev_copies(range(32, 48), nc.gpsimd)
                elif s == 15:
                    group_setup(2)
                    rev_copies(range(16, 32), nc.gpsimd)
                fcol = s * BS               # fwd step s consumes e_t, t = s
                bcol = (SEQ - 1 - s) * BS   # bwd step s consumes e_t, t = 63 - s
                # next-step state tile; e-term refresh copies ride gpsimd
                # (f32 -> f16 cast) entirely off the DVE chain
                S2 = spool.tile([98, BS], dt.float16, tag="S")
                nc.gpsimd.tensor_copy(S2[64:96, :], P20[0:32, fcol + BS : fcol + 2 * BS])
                nc.gpsimd.tensor_copy(S2[96:98, :], P20[32:34, bcol - BS : bcol])

                zrps = zrpsp.tile([128, BS], dt.float32, tag="zr")
                nc.tensor.matmul(zrps[:], lhsT=wzr_sb, rhs=S[:], start=True, stop=True)
                gps = gpsp.tile([64, BS], dt.float32, tag="g")
                nc.tensor.matmul(gps[:], lhsT=whh_sb, rhs=S[0:64, :], start=True, stop=True)
                bc = spool.tile([128, BS], dt.float16, tag="bc")
                nc.scalar.activation(out=bc[:], in_=zrps[:], func=AF.Sigmoid)
                # r-path, in place in PSUM: cand = tanh(r * (Whh.T h) + eh)
                nc.vector.tensor_mul(gps[:], gps[:], bc[64:128, :])
                nc.vector.tensor_add(gps[:], gps[:], P20EH[:, fcol : fcol + BS])
                # z-path (fills the tanh wait): v = h - z*h
                u = spool.tile([64, BS], dt.float16, tag="u")
                nc.vector.tensor_mul(u[:], S[0:64, :], bc[0:64, :])
                v = spool.tile([64, BS], dt.float16, tag="v")
                nc.vector.tensor_sub(v[:], S[0:64, :], u[:])
                cand = spool.tile([64, BS], dt.float16, tag="cand")
                nc.scalar.activation(out=cand[:], in_=gps[:], func=AF.Tanh)
                w = spool.tile([64, BS], dt.float16, tag="w")
                nc.vector.tensor_mul(w[:], cand[:], bc[0:64, :])
                nc.vector.tensor_add(S2[0:64, :], v[:], w[:])
                # pre-update states into the projection lhsT tiles (f16->bf16):
                # fwd block s+1 (rows 0:8 of HTf), bwd block 62-s (HTbk)
                fb = s + 1
                bb = SEQ - 2 - s
                nc.gpsimd.tensor_copy(
                    HTf[fb // 16][0:8, (fb % 16) * BS : (fb % 16) * BS + BS],
                    S2[0:8, :])
                nc.gpsimd.tensor_copy(
                    HTbk[bb // 16][0:8, (bb % 16) * BS : (bb % 16) * BS + BS],
                    S2[32:40, :])
                S = S2

            gpsp.release()
            zrpsp.release()
            pstp.release()

            # assemble compact lhsT: bwd h into rows 8:16 (DMA may cross
            # partition-quadrant boundaries; compute engines may not)
            for m in range(NT):
                nc.sync.dma_start(HTf[m][8:16, :], HTbk[m][:])

            # ---- phase 3: single-pass projection + log_softmax ----
            lpsp = tc.alloc_tile_pool(name="lps", bufs=2, space="PSUM")
            dumpp = tc.alloc_tile_pool(name="dump", bufs=2)

            def dunit(m, u2):
                # 8 bf16 matmuls -> 2 PSUM tiles; two drains into one bf16
                # stage; one exp+accumulate over 4096 staged columns with
                # its out dumped to SBUF scratch (PSUM frees at drain time)
                stg = stgp.tile([128, DCOL], dt.bfloat16, tag="stg")
                for half in range(2):
                    lps = lpsp.tile([128, UCH, NCHUNK], dt.float32, tag="l")
                    for h in range(UCH):
                        j = (2 * u2 + half) * UCH + h
                        nc.tensor.matmul(lps[:, h, :],
                                         lhsT=HTf[m][:],
                                         rhs=wout_sb[:, j * NCHUNK : (j + 1) * NCHUNK],
                                         start=True, stop=True)
                    dst = stg[:, half * UCH * NCHUNK : (half + 1) * UCH * NCHUNK]
                    dst = dst.rearrange("p (f c) -> p f c", f=UCH)
                    if (u2 * 2 + half) % 8 in (2, 5, 7):
                        nc.scalar.copy(dst, lps[:, :, :])
                    else:
                        nc.vector.tensor_copy(dst, lps[:, :, :])
                dump = dumpp.tile([128, DCOL], dt.float32, tag="dump")
                nc.scalar.activation(out=dump[:],
                                     in_=stg[:],
                                     func=AF.Exp,
                                     accum_out=sums[m][:, u2 : u2 + 1])
                return stg

            def nlz_emit(m):
                nc.vector.reduce_sum(out=nlz[m][:, 0:1], in_=sums[m][:, 0:dunits],
                                     axis=mybir.AxisListType.X)
                nc.scalar.activation(out=nlz[m][:, 1:2], in_=nlz[m][:, 0:1], func=AF.Ln)
                nc.vector.tensor_scalar_mul(nlz[m][:, 0:1], nlz[m][:, 1:2], -1.0)

            def final(m, u2, stg):
                # in-place -logsumexp add: all-bf16 SBUF tensor_scalar (fast
                # DVE perf mode), then the output DMA (pad columns dropped)
                nc.vector.tensor_scalar_add(stg[:], stg[:], nlz[m][:, 0:1])
                c0 = u2 * DCOL
                c1 = min((u2 + 1) * DCOL, VOCAB)
                nc.sync.dma_start(
                    out_d[m * 128 : (m + 1) * 128, c0:c1],
                    stg[:, 0 : c1 - c0])

            o = proj_order
            stgs = {}
            for u2 in range(dunits):
                stgs[(o[0], u2)] = dunit(o[0], u2)
            nlz_emit(o[0])
            for k in range(1, NT):
                for u2 in range(dunits):
                    stgs[(o[k], u2)] = dunit(o[k], u2)
                    final(o[k - 1], u2, stgs.pop((o[k - 1], u2)))
                nlz_emit(o[k])
            for u2 in range(dunits):
                final(o[NT - 1], u2, stgs.pop((o[NT - 1], u2)))

            dumpp.release()
            lpsp.release()

    nc.compile()
    return nc


def _prep_weights(embeddings, Wz1, bz1, Wr1, br1, Wh1, bh1, Wz2, bz2, Wr2, br2, Wh2, bh2,
                  Wout, bout):
    f32 = np.float32
    emb = np.ascontiguousarray(np.asarray(embeddings, dtype=f32))
    vocab = emb.shape[0]

    Wz1, Wr1, Wh1 = (np.asarray(a, dtype=f32) for a in (Wz1, Wr1, Wh1))
    Wz2, Wr2, Wh2 = (np.asarray(a, dtype=f32) for a in (Wz2, Wr2, Wh2))

    # We_all [33, 104]: embedding-side weights for all gates, bias row folded
    # in, columns already in the quadrant-aligned P20 row layout:
    # 0=z1, 1=r1, 32=z2, 33=r2, 64:72=h1, 96:104=h2.  cat = [h, e].
    wea = np.zeros((EMB + 1, 104), dtype=f32)
    wea[:EMB, 0] = Wz1[HID:, 0]
    wea[:EMB, 1] = Wr1[HID:, 0]
    wea[:EMB, 32] = Wz2[HID:, 0]
    wea[:EMB, 33] = Wr2[HID:, 0]
    wea[:EMB, 64:72] = Wh1[HID:, :]
    wea[:EMB, 96:104] = Wh2[HID:, :]
    wea[EMB, 0] = np.asarray(bz1)[0]
    wea[EMB, 1] = np.asarray(br1)[0]
    wea[EMB, 32] = np.asarray(bz2)[0]
    wea[EMB, 33] = np.asarray(br2)[0]
    wea[EMB, 64:72] = np.asarray(bh1)
    wea[EMB, 96:104] = np.asarray(bh2)

    # Wzr replicated [98, 128] f16: 32 identical columns per gate so that
    # sigmoid(zr matmul) IS the broadcast gate tile: cols 0:32 = z1,
    # 32:64 = z2, 64:96 = r1, 96:128 = r2.  Selector rows (64=ez1, 65=er1,
    # 96=ez2, 97=er2) pass through the precomputed input-side terms
    # carried in S rows 64:66 / 96:98.
    wzr = np.zeros((98, 128), dtype=f32)
    wzr[0:HID, 0:32] = Wz1[:HID, 0:1]
    wzr[64, 0:32] = 1.0    # ez1
    wzr[32 : 32 + HID, 32:64] = Wz2[:HID, 0:1]
    wzr[96, 32:64] = 1.0   # ez2
    wzr[0:HID, 64:96] = Wr1[:HID, 0:1]
    wzr[65, 64:96] = 1.0   # er1
    wzr[32 : 32 + HID, 96:128] = Wr2[:HID, 0:1]
    wzr[97, 96:128] = 1.0  # er2
    wzrT = np.zeros((128, 128), dtype=np.float16)
    wzrT[:, 0:98] = wzr.T.astype(np.float16)
    wzr = wzrT

    # Whh spread [64, 64] f16: block "diag" hidden-side candidate weights.
    whh = np.zeros((64, 64), dtype=f32)
    whh[0:HID, 0:HID] = Wh1[:HID, :]
    whh[32 : 32 + HID, 32 : 32 + HID] = Wh2[:HID, :]
    whhT = np.zeros((64, 128), dtype=np.float16)
    whhT[:, 0:64] = whh.T.astype(np.float16)
    whh = whhT

    # Compact Wout [17, 32768] bf16: rows 0:8 fwd-h, 8:16 bwd-h, 16 = bout.
    # Pad columns get bias -40 so exp(pad logits) ~ 0 and the padded
    # logsumexp equals the true one.
    Wout = np.asarray(Wout, dtype=f32)
    wout17 = np.zeros((17, VPAD), dtype=f32)
    wout17[0:16, :vocab] = Wout
    wout17[16, :vocab] = np.asarray(bout, dtype=f32)
    wout17[16, vocab:] = -40.0
    wout17 = wout17.astype(ml_dtypes.bfloat16)

    return dict(emb=emb, wea=wea, wzr=wzr, whh=whh, wout=wout17,
                vocab=vocab)


def run(inputs, trace=False):
    from concourse.bass_utils import run_bass_kernel_spmd

    w = _prep_weights(
        inputs["embeddings"],
        inputs["Wz1"], inputs["bz1"], inputs["Wr1"], inputs["br1"],
        inputs["Wh1"], inputs["bh1"],
        inputs["Wz2"], inputs["bz2"], inputs["Wr2"], inputs["br2"],
        inputs["Wh2"], inputs["bh2"],
        inputs["Wout"], inputs["bout"],
    )
    vocab = w.pop("vocab")
    x = np.ascontiguousarray(np.asarray(inputs["x"], dtype=np.int32))
    assert x.shape == (SEQ, BATCH)

    key = ("module", vocab)
    if key not in _module_cache:
        _module_cache[key] = _build_module(vocab=vocab)
    nc = _module_cache[key]

    in_maps = []
    for c in range(NCORES):
        m = dict(w)
        xc = x[:, c * BS : (c + 1) * BS]          # [64, 8]
        # device layout (pre-XBAR): x16[g, p] = xc[g*16 + p//8, p%8], int16
        idx = xc.reshape(TOK // 128, 16, BS).transpose(1, 2, 0).reshape(128, TOK // 128)
        x16 = np.zeros((16, 128), dtype=np.int16)
        x16[0 : TOK // 128, :] = idx.T.astype(np.int16)
        m["x"] = x16
        in_maps.append(m)

    res = run_bass_kernel_spmd(nc, in_maps, core_ids=list(range(NCORES)), trace=trace)
    shards = [res.results[c]["out"].astype(np.float32).reshape(SEQ, BS, vocab)
              for c in range(NCORES)]
    out = np.concatenate(shards, axis=1)
    return out, res


def kernel(**inputs):
    out, _ = run(inputs)
    return out
